# revision 1
# baseline (speedup 1.0000x reference)
"""CGCNN-style GNN message passing on 8 Trainium2 NeuronCores.

Sharding: data-parallel over graphs (4 graphs / core).  Each core holds its
4096 nodes and their 131072 in-edges entirely locally (edges never cross
graph boundaries).  Only BatchNorm batch statistics are all-reduced.

Device-side layout is feature-major ([128 features x nodes/edges]):
  - x tables are kept both feature-major (matmul rhs) and node-major
    "stripes" (gather source: node n -> partition n%128, rank n//128).
  - per-edge pre-activations a = W_col^T x[col] + W_row^T x[row] + w_d*dist
    are accumulated on the PE into PSUM via three matmuls:
      * W_col term: rhs = broadcast AP view of x (col = edge//32, repeated 32x)
      * dist term:  K=1 matmul, rhs = dist row segment
      * W_row term: rhs = gathered x columns (dma_gather transpose=True)
  - gates: sigmoid on ACT; softplus = Ln(1 + Exp(.)) on ACT.
  - message product + 32-edge segment reduction on DVE (tree).
  - BN stats: per-core sum/sumsq -> AllReduce -> affine applied per-partition.
"""

import os
import sys

sys.path.insert(0, "/opt/trn_rl_repo")

import numpy as np
import ml_dtypes

import concourse.bass as bass
import concourse.bacc as bacc
import concourse.mybir as mybir
import concourse.tile as tile

f32 = mybir.dt.float32
bf16 = mybir.dt.bfloat16
i16 = mybir.dt.int16
AF = mybir.ActivationFunctionType
OP = mybir.AluOpType

EPS = 1e-5


class Cfg:
    def __init__(self, G=32, S=1024, DEG=32, D=128, L=4, NCORE=8, CH=8192):
        self.G, self.S, self.DEG, self.D, self.L, self.NCORE = G, S, DEG, D, L, NCORE
        self.GP = G // NCORE            # graphs per core
        self.NN = self.GP * S           # nodes per core
        self.NE = self.NN * DEG         # edges per core
        self.J = self.NE // 128         # edges per partition (dist layout)
        self.CH = min(CH, self.NE)      # edge chunk
        self.NCH = self.NE // self.CH
        self.NT = min(512, self.J)      # matmul N tile
        self.PT = min(1024, self.CH)    # psum tile cols
        assert self.NN % 128 == 0 and self.NE % self.CH == 0
        assert self.PT % self.NT == 0 and self.CH % self.PT == 0
        assert self.NT % DEG == 0
        self.NTOT = self.NN * NCORE     # total nodes (BN denominator)


def wrap16(idx):
    """[n] -> [128, n/16] int16 wrapped layout for dma_gather index tensors."""
    a = np.asarray(idx, np.int16).reshape(-1, 16).T  # [16, n/16]
    return np.tile(a, (8, 1)).copy()                 # [128, n/16]


def build_nc(cfg, debug=False, abl=()):
    # abl: ablation flags for timing experiments: 'nogather','oneact','nomm','nodve'
    NN, NE, D, L, CH, J = cfg.NN, cfg.NE, cfg.D, cfg.L, cfg.CH, cfg.J
    NT, PT, DEG = cfg.NT, cfg.PT, cfg.DEG

    nc = bacc.Bacc("TRN2", target_bir_lowering=False, debug=False,
                   num_devices=cfg.NCORE)

    def din(name, shape, dt):
        return nc.dram_tensor(name, shape, dt, kind="ExternalInput")

    emb_d = din("emb_t", [128, 128], f32)                  # feature-major embT, col z = emb[z]
    zidx_d = din("zidx", [128, NN // 16], i16)
    ridx_d = din("ridx", [128, NE // 16], i16)
    posr_d = din("posr", [128, J * 4], f32)
    posc_d = din("posc", [128, J * 4], f32)
    wcol_d = din("wcol", [L, 2, 128, 128], bf16)
    wrow_d = din("wrow", [L, 2, 128, 128], bf16)
    wdst_d = din("wdst", [L, 2, 1, 128], bf16)
    bias_d = din("bias", [L, 2, 128], f32)
    gc_d = din("gc_p", [L, 128], f32)
    gn_d = din("gn_p", [L, 128], f32)
    bnb_d = din("bnb_p", [L, 128], f32)
    w1_d = din("w1_p", [128, 128], f32)                    # pre-scaled by 1/S
    b1_d = din("b1_p", [128], f32)
    w2_d = din("w2_p", [128, 1], f32)
    b2_d = din("b2_p", [1, 1], f32)
    ident_d = din("ident", [128, 128], bf16)

    out_d = nc.dram_tensor("out4", [1, cfg.GP], f32, kind="ExternalOutput")
    if debug:
        xdbg_d = nc.dram_tensor("xdbg", [128, NN], f32, kind="ExternalOutput")
        adbg_d = nc.dram_tensor("adbg", [128, NN], f32, kind="ExternalOutput")

    groups = [list(range(cfg.NCORE))]

    with tile.TileContext(nc) as tc:
        with (
            tc.tile_pool(name="const", bufs=1) as cp,
            tc.tile_pool(name="xpool", bufs=1) as xp,
            tc.tile_pool(name="node", bufs=1) as npo,
            tc.tile_pool(name="edge", bufs=1) as ep,
            tc.tile_pool(name="idxp", bufs=2) as ip,
            tc.tile_pool(name="gath", bufs=2) as gp,
            tc.tile_pool(name="acts", bufs=1) as ap_,
            tc.tile_pool(name="small", bufs=1) as sp_,
            tc.tile_pool(name="ps", bufs=3, space="PSUM") as pp,
            tc.tile_pool(name="pst", bufs=2, space="PSUM") as ppt,
            tc.tile_pool(name="dram", bufs=2, space="DRAM") as dp,
        ):
            # ---------------- constants ----------------
            emb_sb = cp.tile([128, 128], f32)
            nc.sync.dma_start(emb_sb[:], emb_d[:])
            ident_sb = cp.tile([128, 128], bf16)
            nc.sync.dma_start(ident_sb[:], ident_d[:])
            wcol_sb, wrow_sb, wdst_sb, bias_sb = {}, {}, {}, {}
            for l in range(L):
                for g in range(2):
                    t = cp.tile([128, 128], bf16, tag=f"wc{l}{g}")
                    nc.sync.dma_start(t[:], wcol_d[l, g])
                    wcol_sb[l, g] = t
                    t = cp.tile([128, 128], bf16, tag=f"wr{l}{g}")
                    nc.sync.dma_start(t[:], wrow_d[l, g])
                    wrow_sb[l, g] = t
                    t = cp.tile([1, 128], bf16, tag=f"wd{l}{g}")
                    nc.sync.dma_start(t[:], wdst_d[l, g])
                    wdst_sb[l, g] = t
                    t = cp.tile([128, 1], f32, tag=f"bi{l}{g}")
                    nc.sync.dma_start(t[:], bias_d[l, g].rearrange("(p o) -> p o", o=1))
                    bias_sb[l, g] = t
            gc_sb, gn_sb, bnb_sb = {}, {}, {}
            for l in range(L):
                for nm, d_, dst in (("gc", gc_d, gc_sb), ("gn", gn_d, gn_sb),
                                    ("bb", bnb_d, bnb_sb)):
                    t = cp.tile([128, 1], f32, tag=f"{nm}{l}")
                    nc.sync.dma_start(t[:], d_[l].rearrange("(p o) -> p o", o=1))
                    dst[l] = t
            w1_sb = cp.tile([128, 128], f32)
            nc.sync.dma_start(w1_sb[:], w1_d[:])
            b1_sb = cp.tile([128, 1], f32)
            nc.sync.dma_start(b1_sb[:], b1_d[:].rearrange("(p o) -> p o", o=1))
            w2_sb = cp.tile([128, 1], f32)
            nc.sync.dma_start(w2_sb[:], w2_d[:])
            b2_sb = cp.tile([1, 1], f32)
            nc.sync.dma_start(b2_sb[:], b2_d[:])

            # ---------------- dist (chunked) ----------------
            DQ = max(1, J // 4)
            dist_dram = dp.tile([128, J], bf16, tag="distd")
            for q in range(J // DQ):
                pr = npo.tile([128, DQ * 4], f32, tag="posr")
                pc = npo.tile([128, DQ * 4], f32, tag="posc")
                qs = slice(q * DQ * 4, (q + 1) * DQ * 4)
                nc.sync.dma_start(pr[:], posr_d[:, qs])
                nc.sync.dma_start(pc[:], posc_d[:, qs])
                nc.vector.tensor_sub(out=pr[:], in0=pr[:], in1=pc[:])
                nc.vector.tensor_mul(out=pr[:], in0=pr[:], in1=pr[:])
                dist_f = npo.tile([128, DQ], f32, tag="distf")
                nc.vector.tensor_reduce(
                    out=dist_f[:], in_=pr[:].rearrange("p (j q) -> p j q", q=4),
                    axis=mybir.AxisListType.X, op=OP.add)
                nc.scalar.sqrt(out=dist_f[:], in_=dist_f[:])
                dist_bf = npo.tile([128, DQ], bf16, tag="distb")
                nc.vector.tensor_copy(out=dist_bf[:], in_=dist_f[:])
                nc.sync.dma_start(dist_dram[:, q * DQ:(q + 1) * DQ], dist_bf[:])

            # ---------------- x0 = emb[z], feature-major ----------------
            def make_stripes(src_bf):
                """[128 f, NN] bf16 feature-major -> node-major stripes
                (node n -> partition n%128, byte range [(n//128)*256, +256))."""
                st = xp.tile([128, NN], bf16, tag="stripes")
                for t in range(NN // 128):
                    ptile = ppt.tile([128, 128], bf16, tag="tp")
                    nc.tensor.transpose(ptile[:], src_bf[:, t * 128:(t + 1) * 128],
                                        ident_sb[:])
                    nc.vector.tensor_copy(out=st[:, t * 128:(t + 1) * 128],
                                          in_=ptile[:])
                return st

            xT_f = xp.tile([128, NN], f32, tag="xf32")
            nc.gpsimd.ap_gather(
                xT_f[:].rearrange("p (n d) -> p n d", d=1),
                emb_sb[:].rearrange("p (n d) -> p n d", d=1),
                zidx_d_sb(nc, ip, zidx_d, NN),
                channels=128, num_elems=128, d=1, num_idxs=NN)
            xT_bf = xp.tile([128, NN], bf16, tag="xbf")
            nc.vector.tensor_copy(out=xT_bf[:], in_=xT_f[:])
            stripes = make_stripes(xT_bf)

            NTOT_INV = 1.0 / float(cfg.NTOT)

            # ---------------- layers ----------------
            for l in range(L):
                agg = npo.tile([128, NN], f32, tag="agg")
                for c in range(cfg.NCH):
                    e0c = c * CH
                    idxc = ip.tile([128, CH // 16], i16, tag="idxc")
                    nc.sync.dma_start(
                        idxc[:], ridx_d[:, e0c // 16:(e0c + CH) // 16])
                    sgf = ap_.tile([128, CH], bf16, tag="sgf")
                    usb = ap_.tile([128, CH], bf16, tag="usb")
                    xgc = gp.tile([128, CH], bf16, tag="xg")
                    if "nogather" not in abl:
                        nc.gpsimd.dma_gather(
                            out_ap=xgc[:].rearrange("p (a n) -> p a n", a=1),
                            in_ap=stripes[:], idxs_ap=idxc[:],
                            num_idxs=CH, num_idxs_reg=CH, elem_size=128,
                            transpose=True, sbuf_tokens_per_rank=128,
                            sbuf_free_dim_per_rank=256,
                            sbuf_free_dim_pad_per_rank=0, sbuf_byte_offset=0,
                            single_packet=False)
                    ZR = min(512, PT)           # psum zero-region (f32 elems)
                    for t in range(CH // PT):
                        xg = xgc[:, t * PT:(t + 1) * PT]
                        distc = ip.tile([1, PT], bf16, tag="distc")
                        ed0 = e0c + t * PT
                        nc.sync.dma_start(
                            distc[:],
                            dist_dram[ed0 // J:(ed0 + PT) // J, :]
                            .rearrange("a b -> (a b)")
                            .rearrange("(o n) -> o n", o=1))
                        for g in range(2):
                            ps = pp.tile([128, PT], f32, tag="edge")
                            for r in range(PT // ZR):
                                for wi in range(3):
                                    for u in range(ZR // NT):
                                        ecl = r * ZR + u * NT
                                        e0 = e0c + t * PT + ecl
                                        o = slice(ecl, ecl + NT)
                                        st = (wi == 0 and u == 0)
                                        sp2 = (wi == 2 and u == ZR // NT - 1)
                                        if wi == 0:
                                            n0 = e0 // DEG
                                            nn_ = NT // DEG
                                            rhs = (xT_bf[:, n0:n0 + nn_]
                                                   .unsqueeze(2)
                                                   .to_broadcast((128, nn_, DEG)))
                                            w = wcol_sb[l, g]
                                        elif wi == 1:
                                            rhs = distc[0:1, ecl:ecl + NT]
                                            w = wdst_sb[l, g]
                                        else:
                                            rhs = xg[:, o]
                                            w = wrow_sb[l, g]
                                        if "nomm" in abl and wi != 2:
                                            continue
                                        nc.tensor.matmul(
                                            ps[:, o], w[:], rhs,
                                            start=(st or ("nomm" in abl and u == 0)),
                                            stop=sp2)
                            oc = slice(t * PT, (t + 1) * PT)
                            f2 = AF.Sigmoid if "oneact" in abl else AF.Exp
                            if g == 0:
                                nc.scalar.activation(
                                    out=sgf[:, oc], in_=ps[:], func=AF.Sigmoid,
                                    bias=bias_sb[l, 0][:], scale=1.0)
                            else:
                                nc.scalar.activation(
                                    out=usb[:, oc], in_=ps[:], func=f2,
                                    bias=bias_sb[l, 1][:], scale=1.0)
                    # softplus tail: sp = ln(1 + u)   (in place)
                    if "oneact" not in abl:
                        nc.scalar.activation(out=usb[:], in_=usb[:], func=AF.Ln,
                                             bias=1.0, scale=1.0)
                    # message product (in place into sgf)
                    nc.vector.tensor_mul(out=sgf[:], in0=sgf[:], in1=usb[:])
                    # segment tree-reduce over DEG=32 (5 levels), f32 temps
                    nv = CH // DEG
                    m3 = sgf[:].rearrange("p (n k) -> p n k", k=DEG)
                    t1 = ep.tile([128, nv, 16], bf16, tag="t1")
                    nc.vector.tensor_add(out=t1[:], in0=m3[:, :, 0:16],
                                         in1=m3[:, :, 16:32])
                    t2 = ep.tile([128, nv, 8], f32, tag="t2")
                    nc.vector.tensor_add(out=t2[:], in0=t1[:, :, 0:8],
                                         in1=t1[:, :, 8:16])
                    t3 = ep.tile([128, nv, 4], f32, tag="t3")
                    nc.vector.tensor_add(out=t3[:], in0=t2[:, :, 0:4],
                                         in1=t2[:, :, 4:8])
                    t4 = ep.tile([128, nv, 2], f32, tag="t4")
                    nc.vector.tensor_add(out=t4[:], in0=t3[:, :, 0:2],
                                         in1=t3[:, :, 2:4])
                    nc.vector.tensor_add(
                        out=agg[:, e0c // DEG:e0c // DEG + nv],
                        in0=t4[:, :, 0], in1=t4[:, :, 1])

                # ---- BN1 stats (sum, sumsq over local nodes) + allreduce ----
                def stats_allreduce(src, tagp):
                    NQ = max(1, NN // 4)
                    st = sp_.tile([128, 2], f32, tag=f"st{tagp}")
                    nc.vector.tensor_reduce(out=st[:, 0:1], in_=src[:],
                                            axis=mybir.AxisListType.X, op=OP.add)
                    pq = sp_.tile([128, NN // NQ], f32, tag="sqparts")
                    for q in range(NN // NQ):
                        sc = npo.tile([128, NQ], f32, tag="scratch")
                        qs = slice(q * NQ, (q + 1) * NQ)
                        nc.vector.scalar_tensor_tensor(
                            out=sc[:], in0=src[:, qs], scalar=0.0, in1=src[:, qs],
                            op0=OP.add, op1=OP.mult, accum_out=pq[:, q:q + 1])
                    nc.vector.tensor_reduce(out=st[:, 1:2], in_=pq[:],
                                            axis=mybir.AxisListType.X, op=OP.add)
                    cin = dp.tile([128, 2], f32, tag=f"ci{tagp}")
                    cout = dp.tile([128, 2], f32, tag=f"co{tagp}")
                    nc.sync.dma_start(cin[:], st[:])
                    nc.gpsimd.collective_compute(
                        "AllReduce", OP.add, replica_groups=groups,
                        ins=[cin[:].opt()], outs=[cout[:].opt()])
                    stg = sp_.tile([128, 2], f32, tag=f"sg{tagp}")
                    nc.sync.dma_start(stg[:], cout[:])
                    return stg

                def rsqrt_var(stg, tagp):
                    """stg=[sum,sumsq] -> 1/sqrt(var+eps), and mean."""
                    mu = sp_.tile([128, 1], f32, tag=f"mu{tagp}")
                    nc.vector.tensor_scalar_mul(out=mu[:], in0=stg[:, 0:1],
                                                scalar1=NTOT_INV)
                    msq = sp_.tile([128, 1], f32, tag=f"ms{tagp}")
                    nc.vector.tensor_mul(out=msq[:], in0=mu[:], in1=mu[:])
                    v = sp_.tile([128, 1], f32, tag=f"v{tagp}")
                    nc.vector.scalar_tensor_tensor(
                        out=v[:], in0=stg[:, 1:2], scalar=NTOT_INV, in1=msq[:],
                        op0=OP.mult, op1=OP.subtract)
                    nc.vector.tensor_scalar_add(out=v[:], in0=v[:], scalar1=EPS)
                    s = sp_.tile([128, 1], f32, tag=f"s{tagp}")
                    nc.scalar.sqrt(out=s[:], in_=v[:])
                    r = sp_.tile([128, 1], f32, tag=f"r{tagp}")
                    nc.vector.reciprocal(out=r[:], in_=s[:])
                    # one Newton step on rsqrt: r <- r*(1.5 - 0.5*v*r^2)
                    a = sp_.tile([128, 1], f32, tag=f"a{tagp}")
                    nc.vector.tensor_mul(out=a[:], in0=r[:], in1=r[:])
                    nc.vector.tensor_mul(out=a[:], in0=v[:], in1=a[:])
                    nc.vector.tensor_scalar(out=a[:], in0=a[:], scalar1=-0.5,
                                            scalar2=1.5, op0=OP.mult, op1=OP.add)
                    nc.vector.tensor_mul(out=r[:], in0=r[:], in1=a[:])
                    return r, mu

                stg1 = stats_allreduce(agg, f"1_{l}")
                r1, _mu1 = rsqrt_var(stg1, f"1_{l}")
                gsc = sp_.tile([128, 1], f32, tag=f"gsc{l}")
                nc.vector.tensor_mul(out=gsc[:], in0=gc_sb[l][:], in1=r1[:])
                # x_mid = gsc * agg + x_old   (BN1 shift dropped: cancels in BN2)
                xmid = npo.tile([128, NN], f32, tag="xmid")
                nc.vector.scalar_tensor_tensor(
                    out=xmid[:], in0=agg[:], scalar=gsc[:], in1=xT_f[:],
                    op0=OP.mult, op1=OP.add)

                stg2 = stats_allreduce(xmid, f"2_{l}")
                r2, mu2 = rsqrt_var(stg2, f"2_{l}")
                sc2 = sp_.tile([128, 1], f32, tag=f"sc2{l}")
                nc.vector.tensor_mul(out=sc2[:], in0=gn_sb[l][:], in1=r2[:])
                b2t = sp_.tile([128, 1], f32, tag=f"b2t{l}")
                nc.vector.tensor_mul(out=b2t[:], in0=sc2[:], in1=mu2[:])
                nc.vector.tensor_sub(out=b2t[:], in0=bnb_sb[l][:], in1=b2t[:])
                # x_new = relu(sc2 * x_mid + b2t)
                xT_f = xp.tile([128, NN], f32, tag="xf32")
                nc.scalar.activation(out=xT_f[:], in_=xmid[:], func=AF.Relu,
                                     bias=b2t[:], scale=sc2[:])
                xT_bf = xp.tile([128, NN], bf16, tag="xbf")
                nc.vector.tensor_copy(out=xT_bf[:], in_=xT_f[:])
                if l < L - 1:
                    stripes = make_stripes(xT_bf)

            if debug:
                nc.sync.dma_start(xdbg_d[:], xT_f[:])
                nc.sync.dma_start(adbg_d[:], agg[:])

            # ---------------- readout ----------------
            gsum = sp_.tile([128, cfg.GP], f32, tag="gsum")
            nc.vector.tensor_reduce(
                out=gsum[:], in_=xT_f[:].rearrange("p (g s) -> p g s", s=cfg.S),
                axis=mybir.AxisListType.X, op=OP.add)
            ph = ppt.tile([128, cfg.GP], f32, tag="tp")
            nc.tensor.matmul(ph[:], w1_sb[:], gsum[:], start=True, stop=True)
            h = sp_.tile([128, cfg.GP], f32, tag="h")
            nc.scalar.activation(out=h[:], in_=ph[:], func=AF.Relu,
                                 bias=b1_sb[:], scale=1.0)
            po = ppt.tile([1, cfg.GP], f32, tag="tp")
            nc.tensor.matmul(po[:], w2_sb[:], h[:], start=True, stop=True)
            osb = sp_.tile([1, cfg.GP], f32, tag="osb")
            nc.scalar.activation(out=osb[:], in_=po[:], func=AF.Identity,
                                 bias=b2_sb[:], scale=1.0)
            nc.sync.dma_start(out_d[:], osb[:])

    nc.compile()
    return nc


def zidx_d_sb(nc, ip, zidx_d, NN):
    t = ip.tile([128, NN // 16], i16, tag="zidx")
    nc.sync.dma_start(t[:], zidx_d[:])
    return t[:]


def prep_inputs(inputs, cfg):
    """Full inputs -> per-core input maps (host-side sharding + layout)."""
    bfc = lambda a: np.asarray(a, np.float32).astype(ml_dtypes.bfloat16)
    z = np.asarray(inputs["z"])
    pos = np.asarray(inputs["pos"], np.float32)
    ei = np.asarray(inputs["edge_index"])
    row, col = ei[0].astype(np.int64), ei[1].astype(np.int64)
    Wf = np.asarray(inputs["Wf"], np.float32)
    Ws = np.asarray(inputs["Ws"], np.float32)
    bf_ = np.asarray(inputs["bf"], np.float32)
    bs_ = np.asarray(inputs["bs"], np.float32)
    gc = np.asarray(inputs["gc"], np.float32)
    gn = np.asarray(inputs["gn"], np.float32)
    bnb = np.asarray(inputs["bn_b"], np.float32)
    W1 = np.asarray(inputs["W1"], np.float32)
    b1 = np.asarray(inputs["b1"], np.float32)
    W2 = np.asarray(inputs["W2"], np.float32)
    b2 = np.asarray(inputs["b2"], np.float32)
    emb = np.asarray(inputs["emb"], np.float32)

    D, L = cfg.D, cfg.L
    emb_t = np.zeros((128, 128), np.float32)
    emb_t[:, :emb.shape[0]] = emb.T

    wcol = np.stack([np.stack([bfc(Wf[l, :D]), bfc(Ws[l, :D])]) for l in range(L)])
    wrow = np.stack([np.stack([bfc(Wf[l, D:2 * D]), bfc(Ws[l, D:2 * D])])
                     for l in range(L)])
    wdst = np.stack([np.stack([bfc(Wf[l, 2 * D:2 * D + 1]),
                               bfc(Ws[l, 2 * D:2 * D + 1])]) for l in range(L)])
    biases = np.stack([np.stack([bf_[l], bs_[l]]) for l in range(L)])

    shared = dict(
        emb_t=emb_t, wcol=wcol, wrow=wrow, wdst=wdst, bias=biases,
        gc_p=gc, gn_p=gn, bnb_p=bnb,
        w1_p=(W1 / cfg.S).astype(np.float32),
        b1_p=b1, w2_p=W2, b2_p=b2.reshape(1, 1),
        ident=np.eye(128, dtype=np.float32).astype(ml_dtypes.bfloat16),
    )

    maps = []
    for c in range(cfg.NCORE):
        n0, n1 = c * cfg.NN, (c + 1) * cfg.NN
        e0, e1 = c * cfg.NE, (c + 1) * cfg.NE
        zc = z[n0:n1]
        rl = row[e0:e1] - n0
        assert rl.min() >= 0 and rl.max() < cfg.NN, "edges cross core boundary"
        # per-partition edge layout: partition p owns edges [p*J, (p+1)*J)
        pr = pos[row[e0:e1]]
        pc = pos[col[e0:e1]]
        pr4 = np.zeros((128, cfg.J, 4), np.float32)
        pc4 = np.zeros((128, cfg.J, 4), np.float32)
        pr4[:, :, :3] = pr.reshape(128, cfg.J, 3)
        pc4[:, :, :3] = pc.reshape(128, cfg.J, 3)
        m = dict(shared)
        m.update(
            zidx=wrap16(zc), ridx=wrap16(rl),
            posr=pr4.reshape(128, cfg.J * 4), posc=pc4.reshape(128, cfg.J * 4),
        )
        maps.append(m)
    return maps


_CACHE = {}


def make_runner(nc, n_cores):
    """Build a reusable jitted PJRT executable for `nc` (one NEFF compile +
    load; repeat calls only transfer inputs and execute)."""
    import jax
    from jax.sharding import Mesh, PartitionSpec
    from jax.experimental.shard_map import shard_map
    from concourse.bass2jax import (_bass_exec_p, install_neuronx_cc_hook,
                                    partition_id_tensor)
    import concourse.mybir as mybir

    install_neuronx_cc_hook()
    partition_name = (nc.partition_id_tensor.name
                      if nc.partition_id_tensor else None)
    in_names, out_names, out_avals, zero_outs = [], [], [], []
    for alloc in nc.m.functions[0].allocations:
        if not isinstance(alloc, mybir.MemoryLocationSet):
            continue
        name = alloc.memorylocations[0].name
        if alloc.kind == "ExternalInput":
            if name != partition_name:
                in_names.append(name)
        elif alloc.kind == "ExternalOutput":
            shape = tuple(alloc.tensor_shape)
            dtype = mybir.dt.np(alloc.dtype)
            out_names.append(name)
            out_avals.append(jax.core.ShapedArray(shape, dtype))
            zero_outs.append(np.zeros(shape, dtype))
    n_params = len(in_names)
    n_outs = len(out_avals)
    all_in_names = list(in_names) + list(out_names)
    if partition_name is not None:
        all_in_names.append(partition_name)
    donate = tuple(range(n_params, n_params + n_outs))

    def _body(*args):
        operands = list(args)
        if partition_name is not None:
            operands.append(partition_id_tensor())
        outs = _bass_exec_p.bind(
            *operands, out_avals=tuple(out_avals),
            in_names=tuple(all_in_names), out_names=tuple(out_names),
            lowering_input_output_aliases=(), sim_require_finite=True,
            sim_require_nnan=True, nc=nc)
        return tuple(outs)

    devices = jax.devices()[:n_cores]
    mesh = Mesh(np.asarray(devices), ("core",))
    in_specs = (PartitionSpec("core"),) * (n_params + n_outs)
    out_specs = (PartitionSpec("core"),) * n_outs
    sharded = jax.jit(
        shard_map(_body, mesh=mesh, in_specs=in_specs, out_specs=out_specs,
                  check_rep=False),
        donate_argnums=donate, keep_unused=True)

    def run(maps, device_inputs=None):
        if device_inputs is None:
            device_inputs = stage(maps)
        concat_zeros = [
            np.zeros((n_cores * z.shape[0], *z.shape[1:]), z.dtype)
            for z in zero_outs]
        out_arrs = sharded(*device_inputs, *concat_zeros)
        return [
            {name: np.asarray(out_arrs[i]).reshape(n_cores, *out_avals[i].shape)[c]
             for i, name in enumerate(out_names)}
            for c in range(n_cores)]

    def stage(maps):
        from jax.sharding import NamedSharding
        sh = NamedSharding(mesh, PartitionSpec("core"))
        return [
            jax.device_put(
                np.concatenate([np.asarray(maps[c][nm])
                                for c in range(n_cores)], axis=0), sh)
            for nm in in_names]

    run.stage = stage
    return run


def _get_nc(cfg_key=()):
    if cfg_key not in _CACHE:
        cfg = Cfg()
        nc = build_nc(cfg)
        runner = make_runner(nc, cfg.NCORE)
        _CACHE[cfg_key] = (cfg, nc, runner)
    return _CACHE[cfg_key]


def kernel(**inputs):
    cfg, nc, runner = _get_nc()
    # structural precondition from the generator: edges grouped by target,
    # exactly DEG edges per node
    ei = np.asarray(inputs["edge_index"])
    N = cfg.NN * cfg.NCORE
    assert np.array_equal(ei[1], np.repeat(np.arange(N), cfg.DEG)), \
        "edge_index[1] must be repeat(arange(N), DEG)"
    maps = prep_inputs(inputs, cfg)
    results = runner(maps)
    outs = [results[c]["out4"].reshape(-1) for c in range(cfg.NCORE)]
    return np.concatenate(outs).astype(np.float32)


if __name__ == "__main__":
    cfg = Cfg()
    nc = build_nc(cfg)
    print("built + compiled OK")



# revision 15
# speedup vs baseline: 1.0753x; 1.0753x over previous
"""CGCNN-style GNN message passing on 8 Trainium2 NeuronCores.

Sharding: data-parallel over graphs (4 graphs / core).  Each core holds its
4096 nodes and their 131072 in-edges entirely locally (edges never cross
graph boundaries).  Only BatchNorm batch statistics are all-reduced (one
fused AllReduce per layer carrying [sum_agg, sumsq_agg, sum_x, sumsq_x,
sum_x*agg] so both BN1 and BN2 stats derive from a single collective).

Key structure (per layer):
  - The CGConv linear factors as  ze@W = x[col]@W_col + x[row]@W_row + d*w_d.
    x@W_col and x@W_row have only NN=4096 distinct columns, so we project
    nodes ONCE on the PE (16K columns/layer instead of 786K), write the
    row-projections for both gates into node-major "stripes" (node n ->
    partition n%128, 512B = 256 bf16 = [gate0 feats | gate1 feats]), and
    dma_gather the projected rows per edge.
  - Per edge chunk: PSUM accumulates w_d (x) dist (K=1 matmul) + the col
    term (K=128 matmul with 32x-broadcast rhs); one DVE add merges the
    gathered row projection; ACT applies bias + sigmoid / softplus
    (native); DVE multiplies gates and tree-reduces 32 edges/node.
  - Host ships only true content: compact wrapped indices ([16, n/16],
    replicated to the gather engine's [128, n/16] layout on device),
    host-computed bf16 distances, and the small weights (~1.2MB/core).
"""

import sys

sys.path.insert(0, "/opt/trn_rl_repo")

import numpy as np
import ml_dtypes

import concourse.bass as bass
import concourse.bacc as bacc
import concourse.mybir as mybir
import concourse.tile as tile

f32 = mybir.dt.float32
bf16 = mybir.dt.bfloat16
i16 = mybir.dt.int16
AF = mybir.ActivationFunctionType
OP = mybir.AluOpType

EPS = 1e-5


class Cfg:
    def __init__(self, G=32, S=1024, DEG=32, D=128, L=4, NCORE=8, CH=4096):
        self.G, self.S, self.DEG, self.D, self.L, self.NCORE = G, S, DEG, D, L, NCORE
        self.GP = G // NCORE            # graphs per core
        self.NN = self.GP * S           # nodes per core
        self.NE = self.NN * DEG         # edges per core
        self.CH = CH                    # edge chunk
        self.NCH = self.NE // CH
        self.PT = 1024                  # psum tile cols (edges)
        assert self.NN % 128 == 0 and self.NE % CH == 0 and CH % self.PT == 0
        assert self.NE // 128 == self.PT  # dist rows: one per psum tile
        self.NTOT = self.NN * NCORE     # total nodes (BN denominator)


def build_nc(cfg, debug=False):
    NN, NE, D, L, CH, PT, DEG = cfg.NN, cfg.NE, cfg.D, cfg.L, cfg.CH, cfg.PT, cfg.DEG

    nc = bacc.Bacc("TRN2", target_bir_lowering=False, debug=False,
                   num_devices=cfg.NCORE)

    def din(name, shape, dt):
        return nc.dram_tensor(name, shape, dt, kind="ExternalInput")

    ridx_d = din("ridx", [16, NE // 16], i16)
    zidx_d = din("zidx", [16, NN // 16], i16)
    dist_d = din("dist", [128, NE // 128], bf16)
    emb_d = din("emb_t", [128, 128], f32)                  # feature-major embT
    wcol_d = din("wcol", [L, 2, 128, 128], bf16)
    wrow_d = din("wrow", [L, 2, 128, 128], bf16)
    wdst_d = din("wdst", [L, 2, 1, 128], bf16)
    bias_d = din("bias", [L, 2, 128], f32)
    gc_d = din("gc_p", [L, 128], f32)
    gn_d = din("gn_p", [L, 128], f32)
    bnb_d = din("bnb_p", [L, 128], f32)
    w1_d = din("w1_p", [128, 128], f32)                    # pre-scaled by 1/S
    b1_d = din("b1_p", [128], f32)
    w2_d = din("w2_p", [128, 1], f32)
    b2_d = din("b2_p", [1, 1], f32)
    ident_d = din("ident", [128, 128], bf16)

    out_d = nc.dram_tensor("out4", [1, cfg.GP], f32, kind="ExternalOutput")
    if debug:
        xdbg_d = nc.dram_tensor("xdbg", [128, NN], f32, kind="ExternalOutput")
        adbg_d = nc.dram_tensor("adbg", [128, NN], f32, kind="ExternalOutput")

    groups = [list(range(cfg.NCORE))]
    NTOT_INV = 1.0 / float(cfg.NTOT)

    with tile.TileContext(nc) as tc:
        with (
            tc.tile_pool(name="const", bufs=1) as cp,
            tc.tile_pool(name="xpool", bufs=1) as xp,
            tc.tile_pool(name="node", bufs=1) as npo,
            tc.tile_pool(name="gath", bufs=2) as gp,
            tc.tile_pool(name="acts", bufs=2) as ap_,
            tc.tile_pool(name="edge", bufs=2) as ep,
            tc.tile_pool(name="small", bufs=1) as sp_,
            tc.tile_pool(name="ps", bufs=2, space="PSUM") as pp,
            tc.tile_pool(name="pst", bufs=2, space="PSUM") as ppt,
            tc.tile_pool(name="dram", bufs=2, space="DRAM") as dp,
        ):
            # ---------------- constants ----------------
            emb_sb = cp.tile([128, 128], f32)
            nc.sync.dma_start(emb_sb[:], emb_d[:])
            ident_sb = cp.tile([128, 128], bf16)
            nc.sync.dma_start(ident_sb[:], ident_d[:])
            wcol_sb, wrow_sb, wdst_sb, bias_sb = {}, {}, {}, {}
            for l in range(L):
                for g in range(2):
                    t = cp.tile([128, 128], bf16, tag=f"wc{l}{g}")
                    nc.sync.dma_start(t[:], wcol_d[l, g])
                    wcol_sb[l, g] = t
                    t = cp.tile([128, 128], bf16, tag=f"wr{l}{g}")
                    nc.sync.dma_start(t[:], wrow_d[l, g])
                    wrow_sb[l, g] = t
                    t = cp.tile([1, 128], bf16, tag=f"wd{l}{g}")
                    nc.sync.dma_start(t[:], wdst_d[l, g])
                    wdst_sb[l, g] = t
                    t = cp.tile([128, 1], f32, tag=f"bi{l}{g}")
                    nc.sync.dma_start(t[:], bias_d[l, g].rearrange("(p o) -> p o", o=1))
                    bias_sb[l, g] = t
            gc_sb, gn_sb, bnb_sb = {}, {}, {}
            for l in range(L):
                for nm, d_, dst in (("gc", gc_d, gc_sb), ("gn", gn_d, gn_sb),
                                    ("bb", bnb_d, bnb_sb)):
                    t = cp.tile([128, 1], f32, tag=f"{nm}{l}")
                    nc.sync.dma_start(t[:], d_[l].rearrange("(p o) -> p o", o=1))
                    dst[l] = t
            w1_sb = cp.tile([128, 128], f32)
            nc.sync.dma_start(w1_sb[:], w1_d[:])
            b1_sb = cp.tile([128, 1], f32)
            nc.sync.dma_start(b1_sb[:], b1_d[:].rearrange("(p o) -> p o", o=1))
            w2_sb = cp.tile([128, 1], f32)
            nc.sync.dma_start(w2_sb[:], w2_d[:])
            b2_sb = cp.tile([1, 1], f32)
            nc.sync.dma_start(b2_sb[:], b2_d[:])

            # replicate compact wrapped indices across the 8 gpsimd cores
            ridx_sb = cp.tile([128, NE // 16], i16)
            zidx_sb = cp.tile([128, NN // 16], i16)
            for k in range(8):
                nc.sync.dma_start(ridx_sb[16 * k:16 * (k + 1), :], ridx_d[:])
                nc.sync.dma_start(zidx_sb[16 * k:16 * (k + 1), :], zidx_d[:])

            # ---------------- x0 = emb[z], feature-major ----------------
            xT_f = xp.tile([128, NN], f32, tag="xf32")
            nc.gpsimd.ap_gather(
                xT_f[:].rearrange("p (n d) -> p n d", d=1),
                emb_sb[:].rearrange("p (n d) -> p n d", d=1),
                zidx_sb[:],
                channels=128, num_elems=128, d=1, num_idxs=NN)
            xT_bf = xp.tile([128, NN], bf16, tag="xbf")
            nc.vector.tensor_copy(out=xT_bf[:], in_=xT_f[:])

            # ---------------- layers ----------------
            for l in range(L):
                # ---- x stripes for the per-edge gather: node n ->
                # partition n%128, rank n//128, 256B (128 bf16 features) ----
                stripes = xp.tile([128, NN], bf16, tag="stripes")
                for t in range(NN // 128):
                    pt = ppt.tile([128, 128], bf16, tag="tp")
                    nc.tensor.transpose(pt[:], xT_bf[:, t * 128:(t + 1) * 128],
                                        ident_sb[:])
                    nc.vector.tensor_copy(out=stripes[:, t * 128:(t + 1) * 128],
                                          in_=pt[:])

                # ---- local stats of x (overlap with edge loop) ----
                st = sp_.tile([128, 8], f32, tag="stats")
                NQ = NN // 4
                pq = sp_.tile([128, 4], f32, tag="sqparts")

                def sumsq(src0, src1, dst_col):
                    """dst = sum(src0 * src1) over free axis (f32 accum)."""
                    for q in range(4):
                        sc = npo.tile([128, NQ], f32, tag="scratch")
                        qs = slice(q * NQ, (q + 1) * NQ)
                        nc.vector.scalar_tensor_tensor(
                            out=sc[:], in0=src0[:, qs], scalar=0.0,
                            in1=src1[:, qs], op0=OP.add, op1=OP.mult,
                            accum_out=pq[:, q:q + 1])
                    nc.vector.tensor_reduce(out=st[:, dst_col:dst_col + 1],
                                            in_=pq[:], axis=mybir.AxisListType.X,
                                            op=OP.add)

                nc.vector.tensor_reduce(out=st[:, 2:3], in_=xT_f[:],
                                        axis=mybir.AxisListType.X, op=OP.add)
                sumsq(xT_f, xT_f, 3)

                # ---- edge chunks ----
                agg = npo.tile([128, NN], f32, tag="agg")
                for c in range(cfg.NCH):
                    xg = gp.tile([128, CH], bf16, tag="xg")
                    nc.gpsimd.dma_gather(
                        out_ap=xg[:].rearrange("p (a n) -> p a n", a=1),
                        in_ap=stripes[:],
                        idxs_ap=ridx_sb[:, c * CH // 16:(c + 1) * CH // 16],
                        num_idxs=CH, num_idxs_reg=CH, elem_size=128,
                        transpose=True, sbuf_tokens_per_rank=128,
                        sbuf_free_dim_per_rank=256,
                        sbuf_free_dim_pad_per_rank=0, sbuf_byte_offset=0,
                        single_packet=False)
                    distc = ep.tile([1, CH], bf16, tag="distc")
                    nc.sync.dma_start(
                        distc[:],
                        dist_d[4 * c:4 * (c + 1), :]
                        .rearrange("a b -> (a b)")
                        .rearrange("(o n) -> o n", o=1))
                    sgf = ap_.tile([128, CH], bf16, tag="sgf")
                    usb = ap_.tile([128, CH], bf16, tag="usb")
                    # Gate math: sigma(af) = (1 + tanh(af/2))/2; the /2 is
                    # absorbed by BN1 scale-invariance, so the message is
                    # (1 + tanh(af/2)) * softplus(as) and agg is 2x reference.
                    # Tanh and Exp share an ACT table; Ln is deferred and
                    # batched across chunk pairs -> 1 table load per chunk.
                    for g in range(2):
                        pre = sgf if g == 0 else usb
                        func = AF.Tanh if g == 0 else AF.Exp
                        scl = 0.5 if g == 0 else 1.0    # bias pre-halved host
                        for t in range(CH // PT):
                            n0 = (c * CH + t * PT) // DEG   # first col node
                            ps = pp.tile([128, PT], f32, tag="edge")
                            for u in range(PT // 512):
                                # matmul out must stay within one PSUM bank
                                ou = slice(u * 512, (u + 1) * 512)
                                oe = slice(t * PT + u * 512,
                                           t * PT + (u + 1) * 512)
                                nu = n0 + u * (512 // DEG)
                                nc.tensor.matmul(
                                    ps[:, ou], wdst_sb[l, g][:],
                                    distc[0:1, oe],
                                    start=True, stop=False)
                                nc.tensor.matmul(
                                    ps[:, ou], wcol_sb[l, g][:],
                                    xT_bf[:, nu:nu + 512 // DEG].unsqueeze(2)
                                    .to_broadcast((128, 512 // DEG, DEG)),
                                    start=False, stop=False)
                                nc.tensor.matmul(
                                    ps[:, ou], wrow_sb[l, g][:],
                                    xg[:, oe],
                                    start=False, stop=True)
                            o = slice(t * PT, (t + 1) * PT)
                            nc.scalar.activation(
                                out=pre[:, o], in_=ps[:], func=func,
                                bias=bias_sb[l, g][:], scale=scl)
                    if c % 2 == 0 and c + 1 < cfg.NCH:
                        prev = (sgf, usb)
                        continue
                    # softplus tail (batched over the chunk pair):
                    # usb = ln(1 + exp(as))
                    if c % 2 == 1:
                        psgf, pusb = prev
                        nc.scalar.activation(out=pusb[:], in_=pusb[:],
                                             func=AF.Ln, bias=1.0, scale=1.0)
                    nc.scalar.activation(out=usb[:], in_=usb[:], func=AF.Ln,
                                         bias=1.0, scale=1.0)
                    nv = CH // DEG

                    def reduce_chunk(cc, sgf_, usb_):
                        # msg = (1 + tanh) * softplus, then 32-edge tree sum
                        nc.vector.scalar_tensor_tensor(
                            out=sgf_[:], in0=sgf_[:], scalar=1.0,
                            in1=usb_[:], op0=OP.add, op1=OP.mult)
                        m3 = sgf_[:].rearrange("p (n k) -> p n k", k=DEG)
                        t1 = ep.tile([128, nv, 16], bf16, tag="t1")
                        nc.vector.tensor_add(out=t1[:], in0=m3[:, :, 0:16],
                                             in1=m3[:, :, 16:32])
                        t2 = ep.tile([128, nv, 8], f32, tag="t2")
                        nc.vector.tensor_add(out=t2[:], in0=t1[:, :, 0:8],
                                             in1=t1[:, :, 8:16])
                        t3 = ep.tile([128, nv, 4], f32, tag="t3")
                        nc.vector.tensor_add(out=t3[:], in0=t2[:, :, 0:4],
                                             in1=t2[:, :, 4:8])
                        t4 = ep.tile([128, nv, 2], f32, tag="t4")
                        nc.vector.tensor_add(out=t4[:], in0=t3[:, :, 0:2],
                                             in1=t3[:, :, 2:4])
                        nc.vector.tensor_add(
                            out=agg[:, cc * nv:(cc + 1) * nv],
                            in0=t4[:, :, 0], in1=t4[:, :, 1])

                    if c % 2 == 1:
                        reduce_chunk(c - 1, psgf, pusb)
                    reduce_chunk(c, sgf, usb)

                # ---- fused BN stats: one AllReduce for BN1 + BN2 ----
                nc.vector.tensor_reduce(out=st[:, 0:1], in_=agg[:],
                                        axis=mybir.AxisListType.X, op=OP.add)
                sumsq(agg, agg, 1)
                sumsq(xT_f, agg, 4)
                cin = dp.tile([128, 8], f32, tag=f"ci{l}")
                cout = dp.tile([128, 8], f32, tag=f"co{l}")
                nc.sync.dma_start(cin[:], st[:])
                nc.gpsimd.collective_compute(
                    "AllReduce", OP.add, replica_groups=groups,
                    ins=[cin[:].opt()], outs=[cout[:].opt()])
                stg = sp_.tile([128, 8], f32, tag="statsg")
                nc.sync.dma_start(stg[:], cout[:])

                def rsqrt_of(v, tagp):
                    """v (f32 [128,1]) -> 1/sqrt(v+eps) w/ one Newton step."""
                    nc.vector.tensor_scalar_add(out=v[:], in0=v[:], scalar1=EPS)
                    s = sp_.tile([128, 1], f32, tag=f"s{tagp}")
                    nc.scalar.sqrt(out=s[:], in_=v[:])
                    r = sp_.tile([128, 1], f32, tag=f"r{tagp}")
                    nc.vector.reciprocal(out=r[:], in_=s[:])
                    a = sp_.tile([128, 1], f32, tag=f"a{tagp}")
                    nc.vector.tensor_mul(out=a[:], in0=r[:], in1=r[:])
                    nc.vector.tensor_mul(out=a[:], in0=v[:], in1=a[:])
                    nc.vector.tensor_scalar(out=a[:], in0=a[:], scalar1=-0.5,
                                            scalar2=1.5, op0=OP.mult, op1=OP.add)
                    nc.vector.tensor_mul(out=r[:], in0=r[:], in1=a[:])
                    return r

                # BN1: mu1 = s_agg/N, var1 = q_agg/N - mu1^2, gsc = gc*r1
                mu1 = sp_.tile([128, 1], f32, tag="mu1")
                nc.vector.tensor_scalar_mul(out=mu1[:], in0=stg[:, 0:1],
                                            scalar1=NTOT_INV)
                v1 = sp_.tile([128, 1], f32, tag="v1")
                nc.vector.tensor_mul(out=v1[:], in0=mu1[:], in1=mu1[:])
                nc.vector.scalar_tensor_tensor(
                    out=v1[:], in0=stg[:, 1:2], scalar=NTOT_INV, in1=v1[:],
                    op0=OP.mult, op1=OP.subtract)
                r1 = rsqrt_of(v1, "1")
                gsc = sp_.tile([128, 1], f32, tag="gsc")
                nc.vector.tensor_mul(out=gsc[:], in0=gc_sb[l][:], in1=r1[:])

                # BN2 stats derived: s_mid = gsc*s_agg + s_x
                #                    q_mid = gsc^2*q_agg + 2*gsc*c_xa + q_x
                smid = sp_.tile([128, 1], f32, tag="smid")
                nc.vector.scalar_tensor_tensor(
                    out=smid[:], in0=stg[:, 0:1], scalar=gsc[:], in1=stg[:, 2:3],
                    op0=OP.mult, op1=OP.add)
                qmid = sp_.tile([128, 1], f32, tag="qmid")
                nc.vector.tensor_mul(out=qmid[:], in0=gsc[:], in1=stg[:, 4:5])
                nc.vector.tensor_scalar_mul(out=qmid[:], in0=qmid[:], scalar1=2.0)
                t_b = sp_.tile([128, 1], f32, tag="tmpb")
                nc.vector.tensor_mul(out=t_b[:], in0=gsc[:], in1=gsc[:])
                nc.vector.tensor_mul(out=t_b[:], in0=t_b[:], in1=stg[:, 1:2])
                nc.vector.tensor_add(out=qmid[:], in0=qmid[:], in1=t_b[:])
                nc.vector.tensor_add(out=qmid[:], in0=qmid[:], in1=stg[:, 3:4])

                mu2 = sp_.tile([128, 1], f32, tag="mu2")
                nc.vector.tensor_scalar_mul(out=mu2[:], in0=smid[:],
                                            scalar1=NTOT_INV)
                v2 = sp_.tile([128, 1], f32, tag="v2")
                nc.vector.tensor_mul(out=v2[:], in0=mu2[:], in1=mu2[:])
                nc.vector.scalar_tensor_tensor(
                    out=v2[:], in0=qmid[:], scalar=NTOT_INV, in1=v2[:],
                    op0=OP.mult, op1=OP.subtract)
                r2 = rsqrt_of(v2, "2")
                sc2 = sp_.tile([128, 1], f32, tag="sc2")
                nc.vector.tensor_mul(out=sc2[:], in0=gn_sb[l][:], in1=r2[:])
                b2t = sp_.tile([128, 1], f32, tag="b2t")
                nc.vector.tensor_mul(out=b2t[:], in0=sc2[:], in1=mu2[:])
                nc.vector.tensor_sub(out=b2t[:], in0=bnb_sb[l][:], in1=b2t[:])

                # xmid = gsc*agg + x (BN1 shift dropped: cancels in BN2);
                # x_new = relu(sc2*xmid + b2t)
                nc.vector.scalar_tensor_tensor(
                    out=agg[:], in0=agg[:], scalar=gsc[:], in1=xT_f[:],
                    op0=OP.mult, op1=OP.add)
                xT_f = xp.tile([128, NN], f32, tag="xf32")
                nc.scalar.activation(out=xT_f[:], in_=agg[:], func=AF.Relu,
                                     bias=b2t[:], scale=sc2[:])
                xT_bf = xp.tile([128, NN], bf16, tag="xbf")
                nc.vector.tensor_copy(out=xT_bf[:], in_=xT_f[:])

            if debug:
                nc.sync.dma_start(xdbg_d[:], xT_f[:])
                nc.sync.dma_start(adbg_d[:], agg[:])

            # ---------------- readout ----------------
            gsum = sp_.tile([128, cfg.GP], f32, tag="gsum")
            nc.vector.tensor_reduce(
                out=gsum[:], in_=xT_f[:].rearrange("p (g s) -> p g s", s=cfg.S),
                axis=mybir.AxisListType.X, op=OP.add)
            ph = ppt.tile([128, cfg.GP], f32, tag="tp")
            nc.tensor.matmul(ph[:], w1_sb[:], gsum[:], start=True, stop=True)
            h = sp_.tile([128, cfg.GP], f32, tag="h")
            nc.scalar.activation(out=h[:], in_=ph[:], func=AF.Relu,
                                 bias=b1_sb[:], scale=1.0)
            po = ppt.tile([1, cfg.GP], f32, tag="tp2")
            nc.tensor.matmul(po[:], w2_sb[:], h[:], start=True, stop=True)
            osb = sp_.tile([1, cfg.GP], f32, tag="osb")
            nc.scalar.activation(out=osb[:], in_=po[:], func=AF.Identity,
                                 bias=b2_sb[:], scale=1.0)
            nc.sync.dma_start(out_d[:], osb[:])

    nc.compile()
    return nc


def wrap16(idx):
    """[n] -> [16, n/16] int16 wrapped layout (compact, no replication)."""
    return np.ascontiguousarray(np.asarray(idx, np.int16).reshape(-1, 16).T)


def prep_inputs(inputs, cfg):
    """Full inputs -> per-core input maps (host-side sharding + layout)."""
    bfc = lambda a: np.asarray(a, np.float32).astype(ml_dtypes.bfloat16)
    z = np.asarray(inputs["z"])
    pos = np.asarray(inputs["pos"], np.float32)
    ei = np.asarray(inputs["edge_index"])
    row, col = ei[0], ei[1]
    Wf = np.asarray(inputs["Wf"], np.float32)
    Ws = np.asarray(inputs["Ws"], np.float32)
    bf_ = np.asarray(inputs["bf"], np.float32)
    bs_ = np.asarray(inputs["bs"], np.float32)
    gc = np.asarray(inputs["gc"], np.float32)
    gn = np.asarray(inputs["gn"], np.float32)
    bnb = np.asarray(inputs["bn_b"], np.float32)
    W1 = np.asarray(inputs["W1"], np.float32)
    b1 = np.asarray(inputs["b1"], np.float32)
    W2 = np.asarray(inputs["W2"], np.float32)
    b2 = np.asarray(inputs["b2"], np.float32)
    emb = np.asarray(inputs["emb"], np.float32)

    D, L = cfg.D, cfg.L
    emb_t = np.zeros((128, 128), np.float32)
    emb_t[:, :emb.shape[0]] = emb.T

    wcol = np.stack([np.stack([bfc(Wf[l, :D]), bfc(Ws[l, :D])]) for l in range(L)])
    wrow = np.stack([np.stack([bfc(Wf[l, D:2 * D]), bfc(Ws[l, D:2 * D])])
                     for l in range(L)])
    wdst = np.stack([np.stack([bfc(Wf[l, 2 * D:2 * D + 1]),
                               bfc(Ws[l, 2 * D:2 * D + 1])]) for l in range(L)])
    # gate-0 bias pre-halved: device computes tanh((af + bf)/2) via scale=0.5
    biases = np.stack([np.stack([bf_[l] * 0.5, bs_[l]]) for l in range(L)])

    shared = dict(
        emb_t=emb_t, wcol=wcol, wrow=wrow, wdst=wdst, bias=biases,
        gc_p=gc, gn_p=gn, bnb_p=bnb,
        w1_p=(W1 / cfg.S).astype(np.float32),
        b1_p=b1, w2_p=W2, b2_p=b2.reshape(1, 1),
        ident=np.eye(128, dtype=np.float32).astype(ml_dtypes.bfloat16),
    )

    # per-edge distance on host (edges never cross cores); bf16 is plenty
    # within the 2e-2 gate (it feeds a single matmul input column)
    d = pos[row]
    d.reshape(-1, cfg.DEG, 3)[...] -= pos[:, None, :]   # pos[col]; col = e//DEG
    np.multiply(d, d, out=d)
    s = d[:, 0] + d[:, 1]
    s += d[:, 2]
    np.sqrt(s, out=s)
    dist = s.astype(ml_dtypes.bfloat16)

    rl = (row & (cfg.NN - 1)).astype(np.int16)

    maps = []
    for c in range(cfg.NCORE):
        n0, n1 = c * cfg.NN, (c + 1) * cfg.NN
        e0, e1 = c * cfg.NE, (c + 1) * cfg.NE
        m = dict(shared)
        m.update(
            ridx=wrap16(rl[e0:e1]), zidx=wrap16(z[n0:n1]),
            dist=dist[e0:e1].reshape(128, cfg.NE // 128),
        )
        maps.append(m)
    return maps


_CACHE = {}


def make_runner(nc, n_cores):
    """Build a reusable jitted PJRT executable for `nc` (one NEFF compile +
    load; repeat calls only transfer inputs and execute)."""
    import jax
    from jax.sharding import Mesh, PartitionSpec
    from jax.experimental.shard_map import shard_map
    from concourse.bass2jax import (_bass_exec_p, install_neuronx_cc_hook,
                                    partition_id_tensor)
    import concourse.mybir as mybir

    install_neuronx_cc_hook()
    partition_name = (nc.partition_id_tensor.name
                      if nc.partition_id_tensor else None)
    in_names, out_names, out_avals, zero_outs = [], [], [], []
    for alloc in nc.m.functions[0].allocations:
        if not isinstance(alloc, mybir.MemoryLocationSet):
            continue
        name = alloc.memorylocations[0].name
        if alloc.kind == "ExternalInput":
            if name != partition_name:
                in_names.append(name)
        elif alloc.kind == "ExternalOutput":
            shape = tuple(alloc.tensor_shape)
            dtype = mybir.dt.np(alloc.dtype)
            out_names.append(name)
            out_avals.append(jax.core.ShapedArray(shape, dtype))
            zero_outs.append(np.zeros(shape, dtype))
    n_params = len(in_names)
    n_outs = len(out_avals)
    all_in_names = list(in_names) + list(out_names)
    if partition_name is not None:
        all_in_names.append(partition_name)
    donate = tuple(range(n_params, n_params + n_outs))

    def _body(*args):
        operands = list(args)
        if partition_name is not None:
            operands.append(partition_id_tensor())
        outs = _bass_exec_p.bind(
            *operands, out_avals=tuple(out_avals),
            in_names=tuple(all_in_names), out_names=tuple(out_names),
            lowering_input_output_aliases=(), sim_require_finite=True,
            sim_require_nnan=True, nc=nc)
        return tuple(outs)

    devices = jax.devices()[:n_cores]
    mesh = Mesh(np.asarray(devices), ("core",))
    in_specs = (PartitionSpec("core"),) * (n_params + n_outs)
    out_specs = (PartitionSpec("core"),) * n_outs
    sharded = jax.jit(
        shard_map(_body, mesh=mesh, in_specs=in_specs, out_specs=out_specs,
                  check_rep=False),
        donate_argnums=donate, keep_unused=True)

    def run(maps, device_inputs=None):
        if device_inputs is None:
            device_inputs = stage(maps)
        concat_zeros = [
            np.zeros((n_cores * z.shape[0], *z.shape[1:]), z.dtype)
            for z in zero_outs]
        out_arrs = sharded(*device_inputs, *concat_zeros)
        return [
            {name: np.asarray(out_arrs[i]).reshape(n_cores, *out_avals[i].shape)[c]
             for i, name in enumerate(out_names)}
            for c in range(n_cores)]

    def stage(maps):
        from jax.sharding import NamedSharding
        sh = NamedSharding(mesh, PartitionSpec("core"))
        return [
            jax.device_put(
                np.concatenate([np.asarray(maps[c][nm])
                                for c in range(n_cores)], axis=0), sh)
            for nm in in_names]

    run.stage = stage
    return run


def _get_nc(cfg_key=()):
    if cfg_key not in _CACHE:
        cfg = Cfg()
        nc = build_nc(cfg)
        runner = make_runner(nc, cfg.NCORE)
        _CACHE[cfg_key] = (cfg, nc, runner)
    return _CACHE[cfg_key]


def kernel(**inputs):
    cfg, nc, runner = _get_nc()
    # structural precondition from the generator: edges grouped by target,
    # exactly DEG edges per node, sources within the target's core
    # (sampled checks -- a full scan costs ~100ms on this host)
    ei = np.asarray(inputs["edge_index"])
    N = cfg.NN * cfg.NCORE
    e1v = ei[1].reshape(N, cfg.DEG)
    idx = np.arange(0, N, 97)
    assert (e1v[idx, 0] == idx).all() and (e1v[idx, -1] == idx).all(), \
        "edge_index[1] must be repeat(arange(N), DEG)"
    es = np.arange(0, ei.shape[1], 9973)
    assert (ei[0, es] >> 12 == ei[1, es] >> 12).all(), \
        "edges must not cross core boundaries"
    maps = prep_inputs(inputs, cfg)
    results = runner(maps)
    outs = [results[c]["out4"].reshape(-1) for c in range(cfg.NCORE)]
    return np.concatenate(outs).astype(np.float32)


if __name__ == "__main__":
    cfg = Cfg()
    nc = build_nc(cfg)
    print("built + compiled OK")
